# revision 18
# baseline (speedup 1.0000x reference)
"""3-layer GAT encoder on 8 trn2 NeuronCores (Bass/Tile) - v3.

Sharding: dst-node blocks (core k owns dst nodes [k*6250,(k+1)*6250)); all
segment ops core-local. Aggregation out[n,h] = (sum_e w_e*feat[src_e])/(sum_e
w_e), w_e = exp(leaky_relu(al_s[src]+al_d[dst])).

- Layer 1 gathers nothing: per-edge records (x_src, al_s[src], al_d[dst]) are
  host-known, uploaded dense in block order; the [edge,dst] one-hot is built
  directly on DVE (tensor_tensor is_equal vs an uploaded iota-expanded tile,
  2x mode); no PE transposes, no ald matmuls in layer 1.
- Layers 2/3: [dst,edge] one-hot via TT is_equal against a per-partition iota
  tile (2x); [edge,dst] orientation via PE transposes + scalar-engine copies;
  al_d broadcast by tiny PE matmuls.
- Weighted messages at DVE 2x via scalar-engine-expanded weight tiles; layer 3
  folds weights into the one-hot (single head).
- Node table permuted into two chunks: A = each core's windows 0-24, B =
  windows 25-48. Per layer the AllGather for chunk A fires mid-phase and
  overlaps the rest of the layer; int16 gather indices address each chunk
  table directly.
- K=24 blocks/tile with 3-4 deep pools for pipeline overlap.
"""
import os
import numpy as np
from contextlib import ExitStack

import concourse.bass as bass
import concourse.bacc as bacc
import concourse.tile as tile
from concourse import mybir
from concourse.bass_utils import run_bass_kernel_spmd

F16 = mybir.dt.float16
F32 = mybir.dt.float32
I16 = mybir.dt.int16

N = 50000
NCORE = 8
NLOC = N // NCORE            # 6250
NWIN = (NLOC + 127) // 128   # 49
LASTW = NLOC - 128 * (NWIN - 1)  # 106
WA = 25                      # windows per core in chunk A
RA = WA * 128                # 3200 rows per core in A
RB = NLOC - RA               # 3050 rows per core in B
NTA = NCORE * RA             # 25600 rows table A
NTB = NCORE * RB             # 24400 rows table B
H, F = 3, 43
NEG = 0.2
RECE = 256                   # f16 halves per L2/L3 record (512 B)
DXE = 24                     # f16 halves per L1 dense edge record (48 B)
K = 24                       # blocks per tile


def ap_of(t, offset_elems, dims):
    base = t if isinstance(t, bass.AP) else t[:]
    return bass.AP(tensor=base.tensor, offset=base.offset + offset_elems,
                   ap=[list(base.ap[0])] + [list(d) for d in dims])


def _build_structure(src, dst, x, als1, ald1):
    """Bucket edges by (dst core, dst window, src chunk A/B); uniform block
    structure across cores. Returns per-core upload arrays + schedule."""
    RUNCAP = int(os.environ.get("GAT_RUNCAP", "8"))
    core = dst // NLOC
    dl = dst - core * NLOC
    win = dl // 128
    de = dl - win * 128
    sc = src // NLOC
    sl = src - sc * NLOC
    ch = (sl >= RA).astype(np.int64)
    cidx = np.where(ch == 0, sc * RA + sl, sc * RB + (sl - RA))

    key = (core * NWIN + win) * 2 + ch
    order = np.argsort(key, kind="stable")
    ko = key[order]
    src_o, cidx_o, de_o, dst_o = src[order], cidx[order], de[order], dst[order]
    uniq, starts = np.unique(ko, return_index=True)
    starts = list(starts) + [len(ko)]
    counts = np.zeros((NCORE, NWIN, 2), np.int64)
    seg = {}
    for i, u in enumerate(uniq):
        c_, rem = divmod(int(u), NWIN * 2)
        w_, h_ = divmod(rem, 2)
        s, e = starts[i], starts[i + 1]
        counts[c_, w_, h_] = e - s
        seg[(c_, w_, h_)] = (src_o[s:e], cidx_o[s:e], de_o[s:e], dst_o[s:e])

    B = np.ceil(counts / 128.0).astype(np.int64).max(axis=0)  # [NWIN, 2]
    nb_tot = int(B.sum())
    NT = (nb_tot + K - 1) // K
    B[WA - 1, 0] += NT * K - nb_tot  # pad absorbed into pass-2's last bucket
    nb_tot = NT * K

    # pass 1: all B-chunk blocks (window-major); pass 2: all A-chunk blocks,
    # windows ordered [WA..NWIN-1, 0..WA-1] so the B collective can fire at
    # ~75% of the layer and the A collective at the end.
    order2 = list(range(WA, NWIN)) + list(range(WA))
    blocks = []
    for w_ in range(NWIN):
        blocks += [(w_, 1)] * int(B[w_, 1])
    for w_ in order2:
        blocks += [(w_, 0)] * int(B[w_, 0])
    assert len(blocks) == nb_tot

    sched = []
    prev = None
    for b, (w_, h_) in enumerate(blocks):
        st = (w_, h_) != prev
        sp = (b == nb_tot - 1) or (blocks[b + 1] != (w_, h_))
        sched.append((b // K, b % K, w_, h_, st, sp))
        prev = (w_, h_)
    # tile after which slB (windows WA..NWIN-1) is fully finalized
    TA = max(b for b, (w_, h_) in enumerate(blocks)
             if h_ == 0 and w_ >= WA) // K

    runs = []
    for t in range(NT):
        tb = blocks[t * K:(t + 1) * K]
        i = 0
        while i < len(tb):
            j = i
            while j < len(tb) and tb[j][1] == tb[i][1]:
                j += 1
            for c in range(i, j, RUNCAP):
                runs.append((t, c, min(RUNCAP, j - c), tb[i][1]))
            i = j

    idxw = nb_tot * 8
    idx_cat = np.zeros((NCORE, 128, idxw), np.int16)
    d_e = np.full((NCORE, NT, 128, K), -1.0, np.float16)
    deTe = np.full((NCORE, NT, 128 * K), -1.0, np.float16)
    d_x = np.zeros((NCORE, NT, 128, K * DXE), np.uint16)
    x16 = x.astype(np.float16)
    one16 = np.float16(1.0).view(np.uint16)
    bpos = {}
    bi = 0
    for w_ in range(NWIN):
        bpos[(w_, 1)] = bi
        bi += int(B[w_, 1])
    for w_ in order2:
        bpos[(w_, 0)] = bi
        bi += int(B[w_, 0])
    deTe_v = deTe.reshape(NCORE, NT, 128, K)
    for c_ in range(NCORE):
        for w_ in range(NWIN):
            for h_ in range(2):
                nb = int(B[w_, h_])
                if nb == 0:
                    continue
                s_arr, ci_arr, de_arr, dst_arr = seg.get(
                    (c_, w_, h_), (np.zeros(0, np.int64),) * 4)
                npad = nb * 128 - len(s_arr)
                gsrc = np.concatenate([s_arr, np.zeros(npad, np.int64)])
                gdst = np.concatenate([dst_arr, np.zeros(npad, np.int64)])
                tok = np.concatenate([ci_arr, np.zeros(npad, np.int64)])
                dloc = np.concatenate([de_arr, np.full(npad, -1, np.int64)])
                pmask = np.concatenate([np.ones(len(s_arr), bool),
                                        np.zeros(npad, bool)])
                b0 = bpos[(w_, h_)]
                for b in range(nb):
                    gb = b0 + b
                    t, kk = divmod(gb, K)
                    tk = tok[b * 128:(b + 1) * 128]
                    dd = dloc[b * 128:(b + 1) * 128].astype(np.float16)
                    gs = gsrc[b * 128:(b + 1) * 128]
                    gd = gdst[b * 128:(b + 1) * 128]
                    pm = pmask[b * 128:(b + 1) * 128]
                    d_e[c_, t, :, kk] = dd
                    deTe_v[c_, t, :, kk] = dd
                    wrapped = tk.reshape(8, 16).T.astype(np.int16)  # [16, 8]
                    idx_cat[c_, :, gb * 8:(gb + 1) * 8] = np.tile(wrapped, (8, 1))
                    # L1 record: [x(6) f16, 1, pad, al_s f32x3 @8-13,
                    #             al_d f32x3 @14-19, pad4]
                    rec = np.zeros((128, DXE), np.uint16)
                    rec[:, 0:6] = x16[gs].view(np.uint16)
                    rec[:, 6] = one16
                    rec[:, 8:14] = als1[gs].astype(np.float32).view(
                        np.uint16).reshape(128, 6)
                    rec[:, 14:20] = ald1[gd].astype(np.float32).view(
                        np.uint16).reshape(128, 6)
                    rec[~pm] = 0
                    d_x[c_, t, :, kk * DXE:(kk + 1) * DXE] = rec
    return NT, TA, sched, runs, idx_cat, d_e, deTe, d_x.view(np.float16)


def _build_program(NT, TA, sched, runs, idxw):
    nc = bacc.Bacc("TRN2", target_bir_lowering=False, debug=False,
                   num_devices=NCORE, num_swdge_queues=4)
    de_d = nc.declare_dram_parameter("d_e", [NT, 128, K], F16, isOutput=False)
    deTe_d = nc.declare_dram_parameter("deTe", [NT, 128 * K], F16, isOutput=False)
    dx_d = nc.declare_dram_parameter("d_x", [NT, 128, K * DXE], F16, isOutput=False)
    idx_d = nc.declare_dram_parameter("idx_cat", [128, idxw], I16, isOutput=False)
    iotar_d = nc.declare_dram_parameter("iota_rep", [128 * K], F16, isOutput=False)
    iota_d = nc.declare_dram_parameter("iota32", [128], F32, isOutput=False)
    w0_d = nc.declare_dram_parameter("w0p", [18, 129], F16, isOutput=False)
    we1_d = nc.declare_dram_parameter("wext1", [129, 135], F16, isOutput=False)
    we2_d = nc.declare_dram_parameter("wext2", [129, 130], F16, isOutput=False)
    out_d = nc.declare_dram_parameter("out", [NLOC, 128], F32, isOutput=True)

    slA = {2: nc.dram_tensor("r2sA", [RA * RECE], F16),
           3: nc.dram_tensor("r3sA", [RA * RECE], F16)}
    slB = {2: nc.dram_tensor("r2sB", [RB * RECE], F16),
           3: nc.dram_tensor("r3sB", [RB * RECE], F16)}
    flA = {2: nc.dram_tensor("r2fA", [NTA, RECE], F16, addr_space="Shared"),
           3: nc.dram_tensor("r3fA", [NTA, RECE], F16, addr_space="Shared")}
    flB = {2: nc.dram_tensor("r2fB", [NTB, RECE], F16, addr_space="Shared"),
           3: nc.dram_tensor("r3fB", [NTB, RECE], F16, addr_space="Shared")}

    by_tile = {}
    for (t, kk, w_, h_, st, sp) in sched:
        by_tile.setdefault(t, []).append((kk, w_, h_, st, sp))
    runs_by_tile = {t: [] for t in range(NT)}
    for ri, (t, s, nb, hf) in enumerate(runs):
        runs_by_tile[t].append((ri, s, nb, hf))

    STAGE = int(os.environ.get("GAT_STAGE", "5"))

    with tile.TileContext(nc) as tc, ExitStack() as ctx:
        recs = ctx.enter_context(tc.tile_pool(name="recs", bufs=4))
        ohp = ctx.enter_context(tc.tile_pool(name="ohp", bufs=3))
        pool = ctx.enter_context(tc.tile_pool(name="pool", bufs=3))
        singles = ctx.enter_context(tc.tile_pool(name="singles", bufs=1))
        psums = ctx.enter_context(tc.tile_pool(name="psums", bufs=3, space="PSUM"))
        apsums = ctx.enter_context(tc.tile_pool(name="apsums", bufs=2, space="PSUM"))
        tpsums = ctx.enter_context(tc.tile_pool(name="tpsums", bufs=2, space="PSUM"))
        npsums = ctx.enter_context(tc.tile_pool(name="npsums", bufs=1, space="PSUM"))
        outs = ctx.enter_context(tc.tile_pool(name="outs", bufs=3))

        from concourse.masks import make_identity
        ident = singles.tile([128, 128], F16)
        make_identity(nc, ident[:])
        iota_p = singles.tile([128, 1], F32)
        nc.sync.dma_start(out=iota_p[:], in_=bass.AP(
            tensor=iota_d[:].tensor, offset=0, ap=[[1, 128], [0, 1]]))
        # iota_rep[e, d*K+k] = d (same all partitions) - for L1 e-side one-hot
        iotar_t = singles.tile([128, 128 * K], F16)
        nc.sync.dma_start(out=iotar_t[:], in_=bass.AP(
            tensor=iotar_d[:].tensor, offset=0, ap=[[0, 128], [1, 128 * K]]))
        # iotaP_rep[p, j] = p - for L2/3 d-side one-hot
        iotaP_t = singles.tile([128, 128 * K], F16)
        nc.vector.tensor_copy(out=iotaP_t[:], in_=ap_of(iota_p, 0, [[0, 128 * K]]))
        w0_t = singles.tile([18, 129], F16)
        nc.sync.dma_start(out=w0_t[:], in_=w0_d[:])
        we1_t = singles.tile([128, 135], F16)
        nc.sync.dma_start(out=we1_t[:], in_=we1_d[0:128, :])
        we1b_t = singles.tile([1, 135], F16)
        nc.sync.dma_start(out=we1b_t[:], in_=we1_d[128:129, :])
        we2_t = singles.tile([128, 130], F16)
        nc.sync.dma_start(out=we2_t[:], in_=we2_d[0:128, :])
        we2b_t = singles.tile([1, 130], F16)
        nc.sync.dma_start(out=we2b_t[:], in_=we2_d[128:129, :])

        ald2_t = singles.tile([128, NWIN * H], F16)
        nc.vector.memset(ald2_t[:], 0.0)
        ald3_t = singles.tile([128, NWIN], F16)
        nc.vector.memset(ald3_t[:], 0.0)
        # per-window segment-sum accumulators (pass-1 partials live here)
        acc = [singles.tile([128, 132], F32, tag=f"acc{w}", name=f"acc{w}")
               for w in range(NWIN)]

        def collective(layer, chunk):
            sl = (slA if chunk == 0 else slB)[layer + 1]
            fl = (flA if chunk == 0 else flB)[layer + 1]
            nc.gpsimd.collective_compute(
                "AllGather", mybir.AluOpType.bypass,
                replica_groups=[list(range(NCORE))],
                ins=[sl[:]], outs=[fl[:].rearrange("a b -> (a b)")])

        def edge_phase(layer):
            Hs = H if layer < 3 else 1
            aggw = 21 if layer == 1 else (132 if layer == 2 else 129)
            psum_win = {}
            for t in range(NT):
                if layer == 1:
                    dxt = recs.tile([128, K * DXE], F16, tag="dxt")
                    nc.sync.dma_start(out=dxt[:], in_=dx_d[t])
                    de_t = pool.tile([128, K], F16, tag="de")
                    nc.sync.dma_start(out=de_t[:], in_=de_d[t])
                    # one-hot, edge orientation, direct: oh[e, d*K+k]
                    ohsb = ohp.tile([128, K * 128], F16, tag="ohsb")
                    nc.vector.tensor_tensor(
                        out=ohsb[:],
                        in0=ap_of(de_t, 0, [[0, 128], [1, K]]),
                        in1=iotar_t[:], op=mybir.AluOpType.is_equal)
                else:
                    idx_t = pool.tile([128, K * 8], I16, tag="idx")
                    nc.sync.dma_start(
                        out=idx_t[:], in_=idx_d[:, t * K * 8:(t + 1) * K * 8])
                    rec_t = recs.tile([128, K * RECE], F16, tag="rec")
                    for (ri, s, nb, hf) in runs_by_tile[t]:
                        in_ap = (flA if hf == 0 else flB)[layer][:]
                        base = rec_t[:]
                        out_ap = bass.AP(
                            tensor=base.tensor, offset=base.offset + s * RECE,
                            ap=[list(base.ap[0]), [RECE, nb], [1, RECE]])
                        nc.gpsimd.dma_gather(
                            out_ap=out_ap, in_ap=in_ap,
                            idxs_ap=idx_t[:, s * 8:(s + nb) * 8],
                            num_idxs=nb * 128, num_idxs_reg=nb * 128,
                            elem_size=RECE, queue_num=ri % 4)
                    deTe_b = ohp.tile([128, K * 128], F16, tag="deTe")
                    nc.sync.dma_start(out=deTe_b[:], in_=bass.AP(
                        tensor=deTe_d[:].tensor, offset=t * K * 128,
                        ap=[[0, 128], [1, K * 128]]))
                    # one-hot, dst orientation: oh2[d, e*K+k]
                    oh2_t = ohp.tile([128, K * 128], F16, tag="oh2")
                    nc.vector.tensor_tensor(
                        out=oh2_t[:], in0=deTe_b[:], in1=iotaP_t[:],
                        op=mybir.AluOpType.is_equal)
                    # edge orientation via PE transposes
                    ohsb = ohp.tile([128, K * 128], F16, tag="ohsb")
                    for g in range(K // 8):
                        tps = tpsums.tile([128, 1024], F16, tag="tps", name="tps")
                        for j in range(8):
                            kk = g * 8 + j
                            nc.tensor.transpose(
                                out=tps[:, j * 128:(j + 1) * 128],
                                in_=ap_of(oh2_t, kk, [[K, 128]]),
                                identity=ident[:])
                        nc.scalar.copy(out=ohsb[:, g * 1024:(g + 1) * 1024],
                                       in_=tps[:])
                    # al_d broadcast per edge
                    ald_t = ald2_t if layer == 2 else ald3_t
                    ald_ps = apsums.tile([128, K * Hs], F32, tag="aldps",
                                         name="ald_ps")
                    for (kk, w_, h_, st, sp) in by_tile[t]:
                        nc.tensor.matmul(
                            out=ald_ps[:, kk * Hs:(kk + 1) * Hs],
                            lhsT=ap_of(oh2_t, kk, [[K, 128]]),
                            rhs=ald_t[:, w_ * Hs:(w_ + 1) * Hs],
                            start=True, stop=True)

                # logits -> weights: w = exp(leaky(lg)) = max(exp(lg), exp(.2 lg))
                lg_t = pool.tile([128, K * H], F32, tag="lg")
                wa_t = pool.tile([128, K * H], F16, tag="wa")
                wb_t = pool.tile([128, K * H], F16, tag="wb")
                w_t = pool.tile([128, K * H], F16, tag="w")
                if layer == 1:
                    nc.vector.tensor_add(
                        out=lg_t[:, 0:K * Hs],
                        in0=ap_of(dxt[:].bitcast(F32), 4, [[DXE // 2, K], [1, Hs]]),
                        in1=ap_of(dxt[:].bitcast(F32), 7, [[DXE // 2, K], [1, Hs]]))
                else:
                    als_off = 66 if layer == 2 else 65
                    als_ap = ap_of(rec_t[:].bitcast(F32), als_off,
                                   [[RECE // 2, K], [1, Hs]])
                    nc.vector.tensor_add(out=lg_t[:, 0:K * Hs], in0=als_ap,
                                         in1=ald_ps[:])
                nc.scalar.activation(out=wa_t[:, 0:K * Hs], in_=lg_t[:, 0:K * Hs],
                                     func=mybir.ActivationFunctionType.Exp)
                nc.scalar.activation(out=wb_t[:, 0:K * Hs], in_=lg_t[:, 0:K * Hs],
                                     func=mybir.ActivationFunctionType.Exp,
                                     scale=NEG)
                nc.vector.tensor_max(out=w_t[:, 0:K * Hs], in0=wa_t[:, 0:K * Hs],
                                     in1=wb_t[:, 0:K * Hs])

                # weighted messages
                if layer == 1:
                    rhs1_t = pool.tile([128, K * 21], F16, tag="rhs1")
                    nc.vector.tensor_tensor(
                        out=rhs1_t[:],
                        in0=ap_of(dxt, 0, [[DXE, K], [0, H], [1, 7]]),
                        in1=ap_of(w_t, 0, [[H, K], [1, H], [0, 7]]),
                        op=mybir.AluOpType.mult)
                elif layer == 2:
                    wrep_t = pool.tile([128, K * 132], F16, tag="wrep")
                    nc.scalar.copy(
                        out=wrep_t[:],
                        in_=ap_of(w_t, 0, [[H, K], [1, H], [0, 44]]))
                    nc.vector.tensor_tensor(
                        out=ap_of(rec_t, 0, [[RECE, K], [1, 132]]),
                        in0=ap_of(rec_t, 0, [[RECE, K], [1, 132]]),
                        in1=wrep_t[:], op=mybir.AluOpType.mult)
                else:
                    wrep_t = pool.tile([128, K * 132], F16, tag="wrep")
                    nc.scalar.copy(
                        out=wrep_t[:, 0:K * 128],
                        in_=ap_of(w_t, 0, [[1, K], [0, 128]]))
                    nc.vector.tensor_tensor(
                        out=ohsb[:], in0=ohsb[:], in1=wrep_t[:, 0:K * 128],
                        op=mybir.AluOpType.mult)

                # segment sums per dst window
                for (kk, w_, h_, st, sp) in by_tile[t]:
                    if st:
                        psum_win[w_] = psums.tile([128, aggw], F32,
                                                  tag="agg", name="agg_ps")
                    if layer == 1:
                        rhs = rhs1_t[:, kk * 21:(kk + 1) * 21]
                        lhsT = ap_of(ohsb, kk, [[K, 128]])
                    elif layer == 2:
                        rhs = ap_of(rec_t, kk * RECE, [[1, 132]])
                        lhsT = ohsb[:, kk * 128:(kk + 1) * 128]
                    else:
                        rhs = ap_of(rec_t, kk * RECE, [[1, 129]])
                        lhsT = ohsb[:, kk * 128:(kk + 1) * 128]
                    nc.tensor.matmul(
                        out=psum_win[w_][:], lhsT=lhsT,
                        rhs=rhs, start=st, stop=sp)
                    if sp:
                        ps = psum_win.pop(w_)
                        if h_ == 1:  # pass 1 (B): stash partial
                            nc.vector.tensor_copy(
                                out=acc[w_][:, 0:aggw], in_=ps[:])
                        else:        # pass 2 (A): combine + finalize
                            nc.vector.tensor_add(
                                out=acc[w_][:, 0:aggw],
                                in0=acc[w_][:, 0:aggw], in1=ps[:])
                            finalize(layer, w_, acc[w_])
                if t == TA and layer < 3 and STAGE >= 2:
                    collective(layer, 1)
            if layer < 3 and STAGE >= 2:
                collective(layer, 0)

        def finalize(layer, w_, ps):
            rows = LASTW if w_ == NWIN - 1 else 128
            if layer == 1:
                recip = outs.tile([128, H], F32, tag="recip1")
                nc.vector.reciprocal(out=recip[:], in_=ap_of(ps, 6, [[7, H]]))
                xn_t = outs.tile([128, 18], F16, tag="xn")
                for h in range(H):
                    nc.scalar.activation(
                        out=xn_t[:, 6 * h:6 * h + 6], in_=ps[:, 7 * h:7 * h + 6],
                        func=mybir.ActivationFunctionType.Copy,
                        scale=recip[:, h:h + 1])
                xT_ps = npsums.tile([18, 128], F16, tag="npA", name="xT_ps")
                nc.tensor.transpose(out=xT_ps[:], in_=xn_t[:], identity=ident[:])
                xT_t = outs.tile([18, 128], F16, tag="xTs")
                nc.vector.tensor_copy(out=xT_t[:], in_=xT_ps[:])
                g_ps = npsums.tile([128, 129], F32, tag="npA", name="g1_ps")
                nc.tensor.matmul(out=g_ps[:], lhsT=xT_t[:], rhs=w0_t[:],
                                 start=True, stop=True)
                node_phase(1, w_, g_ps, rows)
            elif layer == 2:
                recip = outs.tile([128, H], F32, tag="recip2")
                nc.vector.reciprocal(out=recip[:], in_=ap_of(ps, 43, [[44, H]]))
                g_t = outs.tile([128, 129], F32, tag="g2pre")
                for h in range(H):
                    nc.scalar.activation(
                        out=g_t[:, F * h:F * h + F], in_=ps[:, 44 * h:44 * h + F],
                        func=mybir.ActivationFunctionType.Copy,
                        scale=recip[:, h:h + 1])
                node_phase(2, w_, g_t, rows)
            else:
                recip = outs.tile([128, 1], F32, tag="recip3")
                nc.vector.reciprocal(out=recip[:], in_=ps[:, 128:129])
                o_t = outs.tile([128, 128], F32, tag="ofin")
                nc.scalar.activation(
                    out=o_t[:], in_=ps[:, 0:128],
                    func=mybir.ActivationFunctionType.Copy, scale=recip[:])
                nc.sync.dma_start(out=out_d[w_ * 128:w_ * 128 + rows, :],
                                  in_=o_t[0:rows, :])

        def node_phase(layer, w_, g_in, rows):
            tmp_t = outs.tile([128, 129], F32, tag="nltmp")
            gl_t = outs.tile([128, 129], F16, tag="nlgl")
            nc.scalar.mul(out=tmp_t[:], in_=g_in[:, 0:129], mul=NEG)
            nc.vector.tensor_max(out=gl_t[:], in0=g_in[:, 0:129], in1=tmp_t[:])
            t01_ps = tpsums.tile([128, 1024], F16, tag="tps", name="t01_ps")
            nc.tensor.transpose(out=t01_ps[:, 0:128], in_=gl_t[:, 0:128], identity=ident[:])
            nc.tensor.transpose(out=t01_ps[0:1, 128:256], in_=gl_t[:, 128:129], identity=ident[:])
            gT0 = outs.tile([128, 128], F16, tag="gT0")
            gT1 = outs.tile([1, 128], F16, tag="gT1")
            nc.scalar.copy(out=gT0[:], in_=t01_ps[:, 0:128])
            nc.scalar.copy(out=gT1[:], in_=t01_ps[0:1, 128:256])
            wa, wb = (we1_t, we1b_t) if layer == 1 else (we2_t, we2b_t)
            wcols = 135 if layer == 1 else 130
            h_ps = npsums.tile([128, wcols], F32, tag="npA", name="h_ps")
            nc.tensor.matmul(out=h_ps[:], lhsT=gT0[:], rhs=wa[:], start=True, stop=False)
            nc.tensor.matmul(out=h_ps[:], lhsT=gT1[:], rhs=wb[:], start=False, stop=True)
            rec_t = outs.tile([128, RECE], F16, tag="recslice")
            nc.vector.memset(rec_t[:], 1.0)
            if layer == 1:
                nc.scalar.copy(
                    out=ap_of(rec_t, 0, [[44, H], [1, F]]),
                    in_=ap_of(h_ps, 0, [[F, H], [1, F]]))
                nc.scalar.copy(
                    out=ap_of(rec_t[:].bitcast(F32), 66, [[1, H]]),
                    in_=h_ps[:, 129:132])
                nc.scalar.copy(out=ald2_t[0:rows, w_ * H:(w_ + 1) * H],
                               in_=h_ps[0:rows, 132:135])
            else:
                nc.scalar.copy(out=rec_t[:, 0:128], in_=h_ps[:, 0:128])
                nc.scalar.copy(
                    out=ap_of(rec_t[:].bitcast(F32), 65, [[1, 1]]),
                    in_=h_ps[:, 128:129])
                nc.scalar.copy(out=ald3_t[0:rows, w_:w_ + 1],
                               in_=h_ps[0:rows, 129:130])
            if w_ < WA:
                dst_dram, roff = (slA[layer + 1], w_ * 128)
            else:
                dst_dram, roff = (slB[layer + 1], (w_ - WA) * 128)
            nc.sync.dma_start(
                out=bass.AP(tensor=dst_dram[:].tensor, offset=roff * RECE,
                            ap=[[RECE, rows], [1, RECE]]),
                in_=rec_t[0:rows, :])

        edge_phase(1)
        if STAGE >= 3:
            edge_phase(2)
        if STAGE >= 5:
            edge_phase(3)

    nc.compile()
    return nc


_CACHE = {}


def run(inputs, trace=False):
    x = np.asarray(inputs["x"], np.float32)
    ei = np.asarray(inputs["edge_index"]).astype(np.int64)
    W0 = np.asarray(inputs["W0"], np.float32)
    a_src0 = np.asarray(inputs["a_src0"], np.float32)
    a_dst0 = np.asarray(inputs["a_dst0"], np.float32)
    b0 = np.asarray(inputs["b0"], np.float32)
    W1 = np.asarray(inputs["W1"], np.float32)
    a_src1 = np.asarray(inputs["a_src1"], np.float32)
    a_dst1 = np.asarray(inputs["a_dst1"], np.float32)
    b1 = np.asarray(inputs["b1"], np.float32)
    W2 = np.asarray(inputs["W2"], np.float32)
    a_src2 = np.asarray(inputs["a_src2"], np.float32)
    a_dst2 = np.asarray(inputs["a_dst2"], np.float32)
    b2 = np.asarray(inputs["b2"], np.float32)
    assert np.all(b0 == 0) and np.all(b1 == 0), "nonzero hidden biases unsupported"

    loops = np.arange(N, dtype=np.int64)
    src = np.concatenate([ei[0], loops])
    dst = np.concatenate([ei[1], loops])

    c_s0 = np.stack([W0[:, h * F:(h + 1) * F] @ a_src0[h] for h in range(H)], 1)
    c_d0 = np.stack([W0[:, h * F:(h + 1) * F] @ a_dst0[h] for h in range(H)], 1)
    al_s1 = x @ c_s0
    al_d1 = x @ c_d0

    skey = hash((src.tobytes(), dst.tobytes(), x.tobytes()))
    if "struct" not in _CACHE or _CACHE.get("skey") != skey:
        _CACHE.update(skey=skey,
                      struct=_build_structure(src, dst, x, al_s1, al_d1))
        _CACHE.pop("nc", None)
    NT, TA, sched, runs, idx_cat, d_e, deTe, d_x = _CACHE["struct"]
    if "nc" not in _CACHE:
        _CACHE["nc"] = _build_program(NT, TA, sched, runs, idx_cat.shape[2])
    nc = _CACHE["nc"]

    def wext(W, a_s, a_d, heads, f):
        cs = np.stack([W[:, h * f:(h + 1) * f] @ a_s[h] for h in range(heads)], 1)
        cd = np.stack([W[:, h * f:(h + 1) * f] @ a_d[h] for h in range(heads)], 1)
        return np.concatenate([W, cs, cd], axis=1).astype(np.float16)

    we1 = wext(W1, a_src1, a_dst1, 3, F)          # [129, 135]
    we2 = wext(W2, a_src2, a_dst2, 1, 128)        # [129, 130]
    w0p = np.zeros((18, 129), np.float16)         # block-diag [3x(6,43)]
    for h in range(H):
        w0p[6 * h:6 * h + 6, F * h:F * (h + 1)] = W0[:, F * h:F * (h + 1)].astype(np.float16)
    iota32 = np.arange(128, dtype=np.float32)
    iota_rep = np.repeat(np.arange(128), K).astype(np.float16)  # [128*K]

    in_maps = []
    for c in range(NCORE):
        in_maps.append(dict(
            d_e=d_e[c], deTe=deTe[c], d_x=d_x[c], idx_cat=idx_cat[c],
            iota_rep=iota_rep, iota32=iota32, w0p=w0p, wext1=we1, wext2=we2))

    res = run_bass_kernel_spmd(nc, in_maps, list(range(NCORE)), trace=trace)
    out = np.concatenate([res.results[c]["out"] for c in range(NCORE)], axis=0)
    out = out + b2[None, :]
    return out.astype(np.float32), res


def kernel(**inputs) -> np.ndarray:
    out, _ = run(inputs, trace=False)
    return out
